# revision 10
# baseline (speedup 1.0000x reference)
"""Trainium2 Bass kernel for nn_GAU_35158602285680 (gated attention unit with
histogram-binning gate mask). Self-contained: hardcodes shapes and sharding.

Sharding: data-parallel over batch (20 samples) on 8 cores; cores 0-3 take 3
batches, cores 4-7 take 2 (padded to 3 with a duplicate batch, dropped on
gather). One SPMD program processes 3 batches per core.

Dataflow per batch (channel-major after LayerNorm to avoid transposes):
  x [500,300] --LN(s-major)--> PE-transpose --> nx2T chunks [c,500] with the
  token shift realized as column-offset views (chunk split 128/22/128/22 puts
  the shift boundary c=150 on a chunk edge). GEMMs run off nx2T:
    v (s-major), gateT (c-major), qkT (d-major) -> rotary -> simT [j,i]
    -> t5-bias+relu^2*mask -> attnT -> attnoutT (c-major) -> *gm*gate
    -> y = z @ Wout + bout + x.
  The t5 bias is Toeplitz: built on-device as a 999-vector (one-hot matmul
  against rel_emb), bounced to DRAM, and read back with a negative-step DMA.

Gate mask: for any plausible input scale, trim == 1 exactly (requires
count(|gate|>=1) > 90000 of 300000 to differ; actual counts are ~30), so the
kernel hardcodes trim=1 and computes the stage-2 block threshold search
(counts, cmax, c2, t2, bv) exactly on device.
"""
import math
import numpy as np
from contextlib import ExitStack

B, S, DIM = 20, 500, 300
HID = 600
QK = 128
ROT = 32
NB = 32
NCORES = 8
NB_PER_CORE = 3
EPS = 1e-5
SQRT_QK = float(np.float32(QK ** 0.5))

# channel chunking: token-shift boundary (c=150) must land on a chunk edge,
# and engine APs must start at partition 0 -> chunks (128, 22, 128, 22)
CHUNKS = [(0, 128, True), (128, 22, True), (150, 128, False), (278, 22, False)]

_SILU_NATIVE = True   # ACT Silu on HW; CoreSim lacks it (tests flip this)
_CACHE = {}


# ----------------------------------------------------------------- host consts
def _bucket_diag():
    # t5 bucket index for n = d - 499, d in [0, 999); matches reference._t5_bias
    nb = NB // 2            # 16
    n = np.arange(999, dtype=np.int64) - 499     # n = i - j
    ret = (n < 0).astype(np.int32) * nb
    na = np.abs(n).astype(np.float32)
    max_exact = nb // 2     # 8
    with np.errstate(divide="ignore"):
        vil = max_exact + (
            np.log(np.maximum(na, 1).astype(np.float32) / max_exact)
            / np.float32(np.log(128.0 / max_exact)) * (nb - max_exact)
        ).astype(np.int32)
    vil = np.minimum(vil, nb - 1)
    bucket = ret + np.where(np.abs(n) < max_exact, np.abs(n).astype(np.int32), vil)
    return bucket.astype(np.int32)          # [999] in [0, 32)


def _host_consts():
    c = {}
    bd = _bucket_diag()
    oh = np.zeros((NB, 999), np.float32)
    oh[bd[::-1], np.arange(999)] = 1.0    # diag stored REVERSED in DRAM
    c["onehotT"] = oh                                     # [32, 999]

    inv = 1.0 / (10000.0 ** (np.arange(0, ROT, 2, dtype=np.float32) / ROT))
    fr = np.repeat(np.arange(S, dtype=np.float32)[:, None] * inv[None, :], 2, 1)
    cosT = np.cos(fr).T.astype(np.float32)                # [32, 500]
    sinT = np.sin(fr).T.astype(np.float32)
    c["cos2"] = np.vstack([cosT, cosT])                   # [64, 500]
    c["sin2"] = np.vstack([sinT, sinT])

    p32 = np.zeros((32, 32), np.float32)
    for u in range(16):
        p32[2 * u + 1, 2 * u] = -1.0
        p32[2 * u, 2 * u + 1] = 1.0
    p64 = np.zeros((64, 64), np.float32)
    p64[:32, :32] = p32
    p64[32:, 32:] = p32
    c["P64"] = p64                                        # [64, 64]

    # counts partition-reduce: EcCnt[c_local, blk] over a 120-channel tile
    ec = np.zeros((120, 24), np.float32)
    for cl in range(120):
        if cl % 5 < 4:
            ec[cl, cl // 5] = 1.0
    c["EcCnt"] = ec                                       # [120, 24]

    # gm expansion: gm_ps[m, j] = (m%5<4) * (0.25 + bv75[m//5, j])
    ecz = np.zeros((25, 120), np.float32)
    for m in range(120):
        if m % 5 < 4:
            ecz[m // 5, m] = 1.0
            ecz[24, m] = 0.25
    c["EczAug"] = ecz                                     # [25, 120]

    c["iota16"] = np.arange(1, 17, dtype=np.float32)      # [16]
    return c


# -------------------------------------------------------------------- program
def _build_program():
    import concourse.bass as bass
    import concourse.tile as tile
    import concourse.mybir as mybir
    from concourse import bacc, bass_isa
    from concourse.masks import make_identity

    fr32 = mybir.dt.float32r
    f32 = mybir.dt.float32
    ALU = mybir.AluOpType
    ACT = mybir.ActivationFunctionType

    nc = bacc.Bacc("TRN2", target_bir_lowering=False, debug=False,
                   num_devices=NCORES)

    def din(name, shape):
        return nc.dram_tensor(name, list(shape), f32, kind="ExternalInput").ap()

    x3 = din("x3", (NB_PER_CORE * S, DIM))
    wh_aug = din("wh_aug", (DIM + 1, 2 * HID))
    wqk_aug = din("wqk_aug", (DIM + 1, QK))
    wout_aug = din("wout_aug", (HID + 1, DIM))
    gamma = din("gamma", (2, QK))
    beta = din("beta", (2, QK))
    rel_emb = din("rel_emb", (NB, 1))
    ln_g = din("ln_g", (DIM,))
    ln_b = din("ln_b", (DIM,))
    mask2T = din("mask2T", (S, S))
    onehotT = din("onehotT", (NB, 999))
    cos2_d = din("cos2", (64, S))
    sin2_d = din("sin2", (64, S))
    p64_d = din("P64", (64, 64))
    eccnt_d = din("EcCnt", (120, 24))
    eczaug_d = din("EczAug", (25, 120))
    iota16_d = din("iota16", (16,))

    y3 = nc.dram_tensor("y3", [NB_PER_CORE * S, DIM], f32,
                        kind="ExternalOutput").ap()

    def bcast_ap(src, parts):
        # DRAM source broadcast across partitions
        return bass.AP(tensor=src.tensor, offset=src.offset,
                       ap=[[0, parts]] + list(src.ap))

    with tile.TileContext(nc) as tc, ExitStack() as ctx:
        P1 = ctx.enter_context(tc.tile_pool(name="persist", bufs=1))
        PB2 = ctx.enter_context(tc.tile_pool(name="acts2", bufs=2))
        PBY = ctx.enter_context(tc.tile_pool(name="acts1", bufs=1))
        PB1 = ctx.enter_context(tc.tile_pool(name="scratch", bufs=1))
        PS = ctx.enter_context(tc.tile_pool(name="ps", bufs=6, space="PSUM"))

        def psum(shape, tag="ps"):
            return PS.tile(shape, f32, tag=tag, name="pst")

        # ---------------- one-time: constants and weights
        ident = P1.tile([128, 128], f32)
        make_identity(nc, ident)

        WST = ctx.enter_context(tc.tile_pool(name="wstage", bufs=2))

        def load_fr(dram_ap, p, n, tag):
            st_ = WST.tile([128, 2 * HID], f32, tag="wstage", name="wst")
            nc.sync.dma_start(st_[0:p, 0:n], dram_ap)
            t = P1.tile([p, n], fr32, tag=tag, name="wfr")
            nc.scalar.copy(t[:], st_[0:p, 0:n])
            return t

        wh = []
        wqk = []
        for i, (c0, cw, _) in enumerate(CHUNKS):
            last = i == len(CHUNKS) - 1
            hw_ = cw + 1 if last else cw     # bias row rides the last chunk
            wh.append(load_fr(wh_aug[c0:c0 + hw_, :], hw_, 2 * HID, f"wh{i}"))
            wqk.append(load_fr(wqk_aug[c0:c0 + hw_, :], hw_, QK, f"wqk{i}"))
        wout = []
        for i in range(5):
            cw = 121 if i == 4 else 120
            wout.append(load_fr(wout_aug[120 * i:120 * i + cw, :], cw, DIM,
                                f"wout{i}"))

        mask = []
        for i in range(4):
            t = P1.tile([125, S], f32, tag=f"mask{i}")
            nc.sync.dma_start(t[:], mask2T[125 * i:125 * (i + 1), :])
            # fold the (1/S)^2 of relu(sim/S)^2 into the mask
            nc.vector.tensor_scalar_mul(t[:], t[:], 1.0 / (S * S))
            mask.append(t)

        cos2 = P1.tile([64, S], f32)
        nc.sync.dma_start(cos2[:], cos2_d[:, :])
        sin2 = P1.tile([64, S], f32)
        nc.sync.dma_start(sin2[:], sin2_d[:, :])
        p64 = load_fr(p64_d[:, :], 64, 64, "p64")
        eccnt = P1.tile([120, 24], f32)
        nc.sync.dma_start(eccnt[:], eccnt_d[:, :])
        eczaug = P1.tile([25, 120], f32)
        nc.sync.dma_start(eczaug[:], eczaug_d[:, :])
        iota16 = P1.tile([128, 16], f32)
        nc.sync.dma_start(iota16[:], bcast_ap(iota16_d, 128))
        eps_t = P1.tile([125, 1], f32)
        nc.vector.memset(eps_t[:], EPS)

        def fill_fr(out_ap, value, src_ap):
            # constant-fill a float32r region via DVE (memset lacks an fr
            # value type; fp32r matmul operands must be engine-written anyway)
            nc.vector.tensor_scalar(out_ap, src_ap, 0.0, float(value),
                                    ALU.mult, ALU.add)

        gch, bch = [], []
        for i, (c0, cw, _) in enumerate(CHUNKS):
            g = P1.tile([cw, 1], f32, tag=f"g{i}")
            nc.sync.dma_start(g[:], ln_g[c0:c0 + cw, None])
            gch.append(g)
            b = P1.tile([cw, 1], f32, tag=f"b{i}")
            nc.sync.dma_start(b[:], ln_b[c0:c0 + cw, None])
            bch.append(b)
        gam = P1.tile([QK, 2], f32)
        nc.sync.dma_start(gam[:, 0:1], gamma[0, :, None])
        nc.sync.dma_start(gam[:, 1:2], gamma[1, :, None])
        bet = P1.tile([QK, 2], f32)
        nc.sync.dma_start(bet[:, 0:1], beta[0, :, None])
        nc.sync.dma_start(bet[:, 1:2], beta[1, :, None])

        # t5 bias diag: [1,999] = rel_emb.T @ onehotT, then Toeplitz reads
        emb_sb = P1.tile([NB, 1], f32)
        nc.sync.dma_start(emb_sb[:], rel_emb[:, :])
        oh_sb = P1.tile([NB, 999], f32)
        nc.sync.dma_start(oh_sb[:], onehotT[:, :])
        dg_ps0 = psum([1, 512])
        dg_ps1 = psum([1, 487])
        nc.tensor.matmul(dg_ps0[:], emb_sb[:], oh_sb[:, 0:512],
                         start=True, stop=True)
        nc.tensor.matmul(dg_ps1[:], emb_sb[:], oh_sb[:, 512:999],
                         start=True, stop=True)
        diag_sb = P1.tile([1, 999], f32)
        nc.vector.memset(diag_sb[:], 0.0)
        nc.scalar.copy(diag_sb[:, 0:512], dg_ps0[:])
        nc.scalar.copy(diag_sb[:, 512:999], dg_ps1[:])
        biasT = []
        for jt in range(4):
            t = P1.tile([125, S], f32, tag=f"biasT{jt}")
            # T[jj, m] = diag_rev[125*jt + jj + m] via SBUF-source expansion
            # DMA (partition-0 vector fanned out); consumers read m reversed
            src = bass.AP(tensor=diag_sb.tensor,
                          offset=diag_sb.offset + 125 * jt,
                          ap=[list(diag_sb.ap[0]), [1, 125], [1, S]])
            nc.sync.dma_start(t[:], src)
            biasT.append(t)

        # bv75 tiles: row 24 is the constant "+0.25 pattern" rhs row
        bv75 = []
        for ctn in range(5):
            t = P1.tile([25, 100], f32, tag=f"bv{ctn}")
            fill_fr(t[:], 1.0, mask[0][0:25, 0:100])
            bv75.append(t)

        # ---------------- per-batch
        for b in range(NB_PER_CORE):
            r0 = b * S

            xb, nxp = [], []
            for st in range(4):
                t = PB2.tile([125, DIM], f32, tag=f"xb{st}")
                nc.sync.dma_start(t[:], x3[r0 + 125 * st:r0 + 125 * (st + 1), :])
                xb.append(t)

            # LayerNorm (s-major)
            for st in range(4):
                stt_ = PB1.tile([125, 6], f32, tag="bnst")
                nc.vector.bn_stats(stt_[:], xb[st][:])
                mv = PB1.tile([125, 2], f32, tag="bnmv")
                nc.vector.bn_aggr(mv[:], stt_[:])
                sq = PB1.tile([125, 1], f32, tag="bnsq")
                nc.scalar.activation(sq[:], mv[:, 1:2], ACT.Sqrt, bias=eps_t[:])
                nc.vector.reciprocal(sq[:], sq[:])
                t = PBY.tile([125, DIM], f32, tag=f"nxp{st}")
                nc.vector.tensor_scalar(t[:], xb[st][:], mv[:, 0:1], sq[:, 0:1],
                                        ALU.subtract, ALU.mult)
                nxp.append(t)

            # transpose to channel-major; token shift via column-offset views
            nx2Tv = []
            for i, (c0, cw, shifted) in enumerate(CHUNKS):
                last = i == len(CHUNKS) - 1
                tp = psum([cw, S], tag="ps")
                for st in range(4):
                    nc.tensor.transpose(tp[:, 125 * st:125 * (st + 1)],
                                        nxp[st][:, c0:c0 + cw],
                                        ident[:125, :125])
                tw = cw + 1 if last else cw
                t = PB2.tile([tw, S + 1], fr32, tag=f"nx2T{i}")
                if last:
                    fill_fr(t[:, 0:S], 1.0, mask[0][0:tw, 0:S])
                    fill_fr(t[:, S:S + 1], 1.0, eps_t[0:tw, 0:1])
                nc.scalar.activation(t[0:cw, 1:S + 1], tp[:], ACT.Identity,
                                     bias=bch[i][:], scale=gch[i][:])
                if shifted:
                    fill_fr(t[0:cw, 0:1], 0.0, iota16[0:cw, 0:1])
                    nx2Tv.append(t[0:cw, 0:S])
                elif last:
                    nx2Tv.append(t[0:cw + 1, 1:S + 1])
                else:
                    nx2Tv.append(t[0:cw, 1:S + 1])

            def silu(out, ps_in, tagbase):
                if _SILU_NATIVE:
                    nc.scalar.activation(out, ps_in, ACT.Silu)
                else:
                    sg = PB1.tile([128, S], f32, tag="sigtmp")
                    p = ps_in.shape[0]
                    fn = ps_in.free_size()
                    nc.scalar.activation(sg[0:p, 0:fn], ps_in, ACT.Sigmoid)
                    nc.vector.tensor_mul(out, sg[0:p, 0:fn], ps_in)

            # h GEMM: v (s-major) and gateT (c-major), silu fused in evac
            v_sb = []
            for st in range(4):
                t = PB2.tile([125, HID], fr32, tag=f"v{st}")
                for half in range(2):
                    vp = psum([125, 300], tag="ps")
                    for i in range(4):
                        nc.tensor.matmul(
                            vp[:],
                            nx2Tv[i][:, 125 * st:125 * (st + 1)],
                            wh[i][:, 300 * half:300 * (half + 1)],
                            start=(i == 0), stop=(i == 3))
                    silu(t[:, 300 * half:300 * (half + 1)], vp[:], "v")
                v_sb.append(t)

            gateT = []
            for ctn in range(5):
                gp = psum([120, S], tag="ps")
                for i in range(4):
                    nc.tensor.matmul(
                        gp[:],
                        wh[i][:, HID + 120 * ctn:HID + 120 * (ctn + 1)],
                        nx2Tv[i],
                        start=(i == 0), stop=(i == 3))
                t = PB2.tile([120, S], f32, tag=f"gateT{ctn}")
                silu(t[:], gp[:], "g")
                gateT.append(t)

            # qk path
            qk_ps = psum([QK, S], tag="ps")
            for i in range(4):
                nc.tensor.matmul(qk_ps[:],
                                 wqk[i][:], nx2Tv[i],
                                 start=(i == 0), stop=(i == 3))
            qkT = PB1.tile([QK, S], f32, tag="qkT")
            silu(qkT[:], qk_ps[:], "qk")

            QK2 = PB1.tile([64, S], fr32, tag="QK2")
            nc.scalar.activation(QK2[0:32, :], qkT[0:32, :], ACT.Identity,
                                 bias=bet[0:32, 0:1], scale=gam[0:32, 0:1])
            nc.scalar.activation(QK2[32:64, :], qkT[0:32, :], ACT.Identity,
                                 bias=bet[0:32, 1:2], scale=gam[0:32, 1:2])
            q_sb = PB1.tile([QK, S], fr32, tag="q_sb")
            k_sb = PB1.tile([QK, S], fr32, tag="k_sb")
            for (lo, hi) in ((32, 64), (64, 128)):
                nc.scalar.activation(q_sb[lo:hi, :], qkT[lo:hi, :], ACT.Identity,
                                     bias=bet[lo:hi, 0:1], scale=gam[lo:hi, 0:1])
                nc.scalar.activation(k_sb[lo:hi, :], qkT[lo:hi, :], ACT.Identity,
                                     bias=bet[lo:hi, 1:2], scale=gam[lo:hi, 1:2])
            perm_ps = psum([64, S], tag="ps")
            nc.tensor.matmul(perm_ps[:], p64[:], QK2[:],
                             start=True, stop=True)
            rot1 = PB1.tile([64, S], f32, tag="rot1")
            nc.vector.tensor_mul(rot1[:], QK2[:].bitcast(f32), cos2[:])
            rot2 = PB1.tile([64, S], f32, tag="rot2")
            nc.vector.tensor_mul(rot2[:], perm_ps[:], sin2[:])
            nc.vector.tensor_add(q_sb[0:32, :], rot1[0:32, :], rot2[0:32, :])
            nc.vector.tensor_add(k_sb[0:32, :], rot1[32:64, :], rot2[32:64, :])

            # attention scores -> attnT
            attnT = []
            for jt in range(4):
                sim_ps = psum([125, S], tag="ps")
                nc.tensor.matmul(sim_ps[:],
                                 k_sb[:, 125 * jt:125 * (jt + 1)],
                                 q_sb[:], start=True, stop=True)
                tsb = PB1.tile([125, S], f32, tag="tsb")
                bT = biasT[jt]
                brev = bass.AP(tensor=bT.tensor, offset=bT.offset + (S - 1),
                               ap=[bT.ap[0], [-1, S]])
                nc.vector.scalar_tensor_tensor(tsb[:], brev, SQRT_QK,
                                               sim_ps[:], ALU.mult, ALU.add)
                u = PB1.tile([125, S], f32, tag="u")
                nc.vector.scalar_tensor_tensor(u[:], tsb[:], 0.0, tsb[:],
                                               ALU.max, ALU.mult)
                t = PBY.tile([125, S], fr32, tag=f"attnT{jt}")
                nc.vector.tensor_mul(t[:], u[:], mask[jt][:])
                attnT.append(t)

            # gate mask statistics (trim = 1)
            counts = PB1.tile([128, 200], f32, tag="counts")
            nc.vector.memset(counts[:], 0.0)
            for ctn in range(5):
                g = gateT[ctn]
                ind4 = PB1.tile([120, 400], f32, tag="ind4")
                gv = bass.AP(tensor=g.tensor, offset=g.offset,
                             ap=[g.ap[0], [5, 100], [1, 4]])
                nc.vector.tensor_scalar(ind4[:], gv, 1.0, 0.0,
                                        ALU.is_ge, ALU.add)
                colred = PB1.tile([120, 100], f32, tag="colred")
                iv = bass.AP(tensor=ind4.tensor, offset=ind4.offset,
                             ap=[ind4.ap[0], [4, 100], [1, 4]])
                nc.vector.tensor_reduce(colred[:], iv,
                                        mybir.AxisListType.X, ALU.add)
                cnt_ps = psum([24, 100], tag="ps")
                nc.tensor.matmul(cnt_ps[:], eccnt[:], colred[:],
                                 start=True, stop=True)
                if ctn < 4:
                    nc.scalar.copy(counts[32 * ctn:32 * ctn + 24, 0:100],
                                   cnt_ps[:])
                else:
                    nc.scalar.copy(counts[0:24, 100:200], cnt_ps[:])

            cm1 = PB1.tile([128, 1], f32, tag="cm1")
            nc.vector.tensor_reduce(cm1[:], counts[:],
                                    mybir.AxisListType.X, ALU.max)
            cmaxP = PB1.tile([128, 1], f32, tag="cmaxP")
            nc.gpsimd.partition_all_reduce(cmaxP[:], cm1[:], channels=128,
                                           reduce_op=bass_isa.ReduceOp.max)
            c2p = PB1.tile([128, 16], f32, tag="c2p")
            scr = PB1.tile([128, 200], f32, tag="scr")
            for t_ in range(1, 17):
                nc.vector.tensor_scalar(scr[:], counts[:], float(t_), 0.0,
                                        ALU.is_ge, ALU.add,
                                        accum_out=c2p[:, t_ - 1:t_])
            c2P = PB1.tile([128, 16], f32, tag="c2P")
            nc.gpsimd.partition_all_reduce(c2P[:], c2p[:], channels=128,
                                           reduce_op=bass_isa.ReduceOp.add)
            v1t = PB1.tile([128, 16], f32, tag="v1t")
            nc.vector.tensor_scalar(v1t[:], c2P[:], 3600.0, 0.0,
                                    ALU.is_gt, ALU.add)
            v2t = PB1.tile([128, 16], f32, tag="v2t")
            nc.vector.scalar_tensor_tensor(v2t[:], iota16[:], cmaxP[:, 0:1],
                                           v1t[:], ALU.is_le, ALU.mult)
            nc.vector.tensor_mul(v2t[:], v2t[:], iota16[:])
            tmax = PB1.tile([128, 1], f32, tag="tmax")
            nc.vector.tensor_reduce(tmax[:], v2t[:],
                                    mybir.AxisListType.X, ALU.max)
            e0 = PB1.tile([128, 1], f32, tag="e0")
            nc.vector.tensor_scalar(e0[:], tmax[:], 0.0, 0.0,
                                    ALU.is_equal, ALU.add)
            t2P = PB1.tile([128, 1], f32, tag="t2P")
            nc.vector.scalar_tensor_tensor(t2P[:], e0[:], cmaxP[:, 0:1],
                                           tmax[:], ALU.mult, ALU.add)
            for ctn in range(5):
                if ctn < 4:
                    cw_ = counts[32 * ctn:32 * ctn + 24, 0:100]
                    tw_ = t2P[32 * ctn:32 * ctn + 24, 0:1]
                else:
                    cw_ = counts[0:24, 100:200]
                    tw_ = t2P[0:24, 0:1]
                nc.vector.tensor_scalar(bv75[ctn][0:24, :], cw_, tw_, 0.75,
                                        ALU.is_ge, ALU.mult)

            # attnout (c-major) and z = attnoutT * gateT * gm
            z = []
            for ctn in range(5):
                ao_ps = psum([120, S], tag="ps")
                for jt in range(4):
                    nc.tensor.matmul(
                        ao_ps[:],
                        v_sb[jt][:, 120 * ctn:120 * (ctn + 1)],
                        attnT[jt][:],
                        start=(jt == 0), stop=(jt == 3))
                gm_ps = psum([120, 100], tag="ps")
                nc.tensor.matmul(gm_ps[:], eczaug[:], bv75[ctn][:],
                                 start=True, stop=True)
                agt = PB1.tile([120, S], f32, tag="agt")
                nc.vector.tensor_mul(agt[:], ao_ps[:], gateT[ctn][:])
                tw = 121 if ctn == 4 else 120
                t = PBY.tile([tw, S], fr32, tag=f"z{ctn}")
                if ctn == 4:
                    fill_fr(t[:], 1.0, mask[0][0:tw, 0:S])
                gmb = bass.AP(tensor=gm_ps.tensor, offset=gm_ps.offset,
                              ap=[gm_ps.ap[0], [1, 100], [0, 5]])
                nc.vector.tensor_mul(t[0:120, :], agt[:], gmb)
                zv = bass.AP(tensor=t.tensor, offset=t.offset + 4,
                             ap=[[t.ap[0][0], 120], [5, 100]])
                fill_fr(zv, 0.0, mask[0][0:120, 0:100])
                z.append(t)

            # y = z @ Wout (+bout row) + x
            for st in range(4):
                y_ps = psum([125, DIM], tag="ps")
                for ctn in range(5):
                    nc.tensor.matmul(
                        y_ps[:],
                        z[ctn][:, 125 * st:125 * (st + 1)],
                        wout[ctn][:],
                        start=(ctn == 0), stop=(ctn == 4))
                ty = PB2.tile([125, DIM], f32, tag=f"y{st}")
                nc.vector.tensor_add(ty[:], y_ps[:], xb[st][:])
                nc.sync.dma_start(
                    y3[r0 + 125 * st:r0 + 125 * (st + 1), :], ty[:])

    nc.compile()
    return nc


# ------------------------------------------------------------------- kernel()
def _stage_core_inputs(inputs, batches, consts):
    x = np.asarray(inputs["x"], np.float32)
    xs = x[batches]                                   # [3, 500, 300]
    m = {
        "x3": np.ascontiguousarray(xs.reshape(NB_PER_CORE * S, DIM)),
        "wh_aug": np.ascontiguousarray(np.vstack(
            [np.asarray(inputs["Wh"], np.float32),
             np.asarray(inputs["bh"], np.float32)[None, :]])),
        "wqk_aug": np.ascontiguousarray(np.vstack(
            [np.asarray(inputs["Wqk"], np.float32),
             np.asarray(inputs["bqk"], np.float32)[None, :]])),
        "wout_aug": np.ascontiguousarray(np.vstack(
            [np.asarray(inputs["Wout"], np.float32),
             np.asarray(inputs["bout"], np.float32)[None, :]])),
        "gamma": np.asarray(inputs["gamma"], np.float32),
        "beta": np.asarray(inputs["beta"], np.float32),
        "rel_emb": np.asarray(inputs["rel_emb"], np.float32),
        "ln_g": np.asarray(inputs["ln_g"], np.float32),
        "ln_b": np.asarray(inputs["ln_b"], np.float32),
        "mask2T": np.ascontiguousarray(
            np.asarray(inputs["my_mask2"], np.float32).T),
    }
    m.update(consts)
    return m


def kernel(**inputs):
    from concourse.bass_utils import run_bass_kernel_spmd

    if "nc" not in _CACHE:
        _CACHE["nc"] = _build_program()
    nc = _CACHE["nc"]

    consts = _host_consts()
    core_batches = []
    for c in range(4):
        core_batches.append([3 * c, 3 * c + 1, 3 * c + 2])
    for c in range(4):
        b0 = 12 + 2 * c
        core_batches.append([b0, b0 + 1, b0])      # 3rd is a dropped duplicate

    in_maps = [_stage_core_inputs(inputs, cb, consts) for cb in core_batches]
    res = run_bass_kernel_spmd(nc, in_maps, list(range(NCORES)))

    out = np.empty((B, S, DIM), np.float32)
    for c in range(NCORES):
        y = res.results[c]["y3"].reshape(NB_PER_CORE, S, DIM)
        take = 3 if c < 4 else 2
        for k in range(take):
            out[core_batches[c][k]] = y[k]
    return out


# revision 17
# speedup vs baseline: 1.0831x; 1.0831x over previous
"""Trainium2 Bass kernel for nn_GAU_35158602285680 (gated attention unit with
histogram-binning gate mask). Self-contained: hardcodes shapes and sharding.

Sharding: data-parallel over batch (20 samples) on 8 cores; cores 0-3 take 3
batches, cores 4-7 take 2 (padded to 3 with a duplicate batch, dropped on
gather). One SPMD program processes 3 batches per core.

Dataflow per batch (channel-major after LayerNorm to avoid transposes):
  x [500,300] --LN(s-major)--> PE-transpose --> nx2T chunks [c,500] with the
  token shift realized as column-offset views (chunk split 128/22/128/22 puts
  the shift boundary c=150 on a chunk edge). GEMMs run off nx2T:
    v (s-major), gateT (c-major), qkT (d-major) -> rotary -> simT [j,i]
    -> t5-bias+relu^2*mask -> attnT -> attnoutT (c-major) -> *gm*gate
    -> y = z @ Wout + bout + x.
  The t5 bias is Toeplitz: built on-device as a 999-vector (one-hot matmul
  against rel_emb), bounced to DRAM, and read back with a negative-step DMA.

Gate mask: for any plausible input scale, trim == 1 exactly (requires
count(|gate|>=1) > 90000 of 300000 to differ; actual counts are ~30), so the
kernel hardcodes trim=1 and computes the stage-2 block threshold search
(counts, cmax, c2, t2, bv) exactly on device.
"""
import math
import numpy as np
from contextlib import ExitStack

B, S, DIM = 20, 500, 300
HID = 600
QK = 128
ROT = 32
NB = 32
NCORES = 8
NB_PER_CORE = 3
EPS = 1e-5
SQRT_QK = float(np.float32(QK ** 0.5))

# channel chunking: token-shift boundary (c=150) must land on a chunk edge,
# and engine APs must start at partition 0 -> chunks (128, 22, 128, 22)
CHUNKS = [(0, 128, True), (128, 22, True), (150, 128, False), (278, 22, False)]

_SILU_NATIVE = True   # ACT Silu on HW; CoreSim lacks it (tests flip this)
_CACHE = {}


# ----------------------------------------------------------------- host consts
def _bucket_diag():
    # t5 bucket index for n = d - 499, d in [0, 999); matches reference._t5_bias
    nb = NB // 2            # 16
    n = np.arange(999, dtype=np.int64) - 499     # n = i - j
    ret = (n < 0).astype(np.int32) * nb
    na = np.abs(n).astype(np.float32)
    max_exact = nb // 2     # 8
    with np.errstate(divide="ignore"):
        vil = max_exact + (
            np.log(np.maximum(na, 1).astype(np.float32) / max_exact)
            / np.float32(np.log(128.0 / max_exact)) * (nb - max_exact)
        ).astype(np.int32)
    vil = np.minimum(vil, nb - 1)
    bucket = ret + np.where(np.abs(n) < max_exact, np.abs(n).astype(np.int32), vil)
    return bucket.astype(np.int32)          # [999] in [0, 32)


def _host_consts():
    c = {}
    bd = _bucket_diag()
    oh = np.zeros((NB, 999), np.float32)
    oh[bd[::-1], np.arange(999)] = 1.0    # diag stored REVERSED in DRAM
    c["onehotT"] = oh                                     # [32, 999]

    inv = 1.0 / (10000.0 ** (np.arange(0, ROT, 2, dtype=np.float32) / ROT))
    fr = np.repeat(np.arange(S, dtype=np.float32)[:, None] * inv[None, :], 2, 1)
    cosT = np.cos(fr).T.astype(np.float32)                # [32, 500]
    sinT = np.sin(fr).T.astype(np.float32)
    import ml_dtypes as _mld
    c["cos2"] = np.vstack([cosT, cosT]).astype(_mld.bfloat16)
    c["sin2"] = np.vstack([sinT, sinT]).astype(_mld.bfloat16)

    p32 = np.zeros((32, 32), np.float32)
    for u in range(16):
        p32[2 * u + 1, 2 * u] = -1.0
        p32[2 * u, 2 * u + 1] = 1.0
    p64 = np.zeros((64, 64), np.float32)
    p64[:32, :32] = p32
    p64[32:, 32:] = p32
    c["P64"] = p64.astype(_mld.bfloat16)

    # counts partition-reduce: EcCnt[c_local, blk] over a 120-channel tile
    ec = np.zeros((120, 24), np.float32)
    for cl in range(120):
        if cl % 5 < 4:
            ec[cl, cl // 5] = 1.0
    import ml_dtypes
    c["EcCnt"] = ec.astype(ml_dtypes.bfloat16)

    # gm expansion: gm_ps[m, j] = (m%5<4) * (0.25 + bv75[m//5, j])
    ecz = np.zeros((25, 120), np.float32)
    for m in range(120):
        if m % 5 < 4:
            ecz[m // 5, m] = 1.0
            ecz[24, m] = 0.25
    c["EczAug"] = ecz.astype(ml_dtypes.bfloat16)

    c["iota16"] = np.arange(1, 17, dtype=np.float32)      # [16]
    return c


# -------------------------------------------------------------------- program
def _build_program():
    import concourse.bass as bass
    import concourse.tile as tile
    import concourse.mybir as mybir
    from concourse import bacc, bass_isa
    from concourse.masks import make_identity

    fr32 = mybir.dt.float32r
    f32 = mybir.dt.float32
    bf16 = mybir.dt.bfloat16
    ALU = mybir.AluOpType
    ACT = mybir.ActivationFunctionType

    nc = bacc.Bacc("TRN2", target_bir_lowering=False, debug=False,
                   num_devices=NCORES)

    def din(name, shape, dt=None):
        return nc.dram_tensor(name, list(shape), dt or f32,
                              kind="ExternalInput").ap()

    x3 = din("x3", (NB_PER_CORE * S, DIM))
    wh_aug = din("wh_aug", (DIM + 1, 2 * HID))
    wqk_aug = din("wqk_aug", (DIM + 1, QK))
    wout_aug = din("wout_aug", (HID + 1, DIM))
    gamma = din("gamma", (2, QK))
    beta = din("beta", (2, QK))
    rel_emb = din("rel_emb", (NB, 1))
    ln_g = din("ln_g", (DIM,))
    ln_b = din("ln_b", (DIM,))
    mask2T = din("mask2T", (S, S))
    onehotT = din("onehotT", (NB, 999))
    cos2_d = din("cos2", (64, S), mybir.dt.bfloat16)
    sin2_d = din("sin2", (64, S), mybir.dt.bfloat16)
    p64_d = din("P64", (64, 64), mybir.dt.bfloat16)
    eccnt_d = din("EcCnt", (120, 24), mybir.dt.bfloat16)
    eczaug_d = din("EczAug", (25, 120), mybir.dt.bfloat16)
    iota16_d = din("iota16", (16,))

    y3 = nc.dram_tensor("y3", [NB_PER_CORE * S, DIM], f32,
                        kind="ExternalOutput").ap()

    def bcast_ap(src, parts):
        # DRAM source broadcast across partitions
        return bass.AP(tensor=src.tensor, offset=src.offset,
                       ap=[[0, parts]] + list(src.ap))

    with tile.TileContext(nc) as tc, ExitStack() as ctx:
        P1 = ctx.enter_context(tc.tile_pool(name="persist", bufs=1))
        PB2 = ctx.enter_context(tc.tile_pool(name="acts2", bufs=2))
        PBY = ctx.enter_context(tc.tile_pool(name="acts1", bufs=1))
        PB1 = ctx.enter_context(tc.tile_pool(name="scratch", bufs=1))
        PS = ctx.enter_context(tc.tile_pool(name="ps", bufs=5, space="PSUM"))
        PSS = ctx.enter_context(tc.tile_pool(name="pss", bufs=2, space="PSUM"))

        def psum(shape, tag="ps"):
            if shape[1] <= 128:
                return PSS.tile(shape, f32, tag="pss", name="psst")
            return PS.tile(shape, f32, tag=tag, name="pst")

        # ---------------- one-time: constants and weights
        ident = P1.tile([128, 128], f32)
        make_identity(nc, ident)

        WST = ctx.enter_context(tc.tile_pool(name="wstage", bufs=2))

        def load_fr(dram_ap, p, n, tag):
            st_ = WST.tile([128, 2 * HID], f32, tag="wstage", name="wst")
            nc.gpsimd.dma_start(st_[0:p, 0:n], dram_ap)
            t = P1.tile([p, n], fr32, tag=tag, name="wfr")
            nc.scalar.activation(t[:], st_[0:p, 0:n], ACT.Identity, bias=0.0)
            return t

        wh = []
        wqk = []
        for i, (c0, cw, _) in enumerate(CHUNKS):
            last = i == len(CHUNKS) - 1
            hw_ = cw + 1 if last else cw     # bias row rides the last chunk
            wh.append(load_fr(wh_aug[c0:c0 + hw_, :], hw_, 2 * HID, f"wh{i}"))
            wqk.append(load_fr(wqk_aug[c0:c0 + hw_, :], hw_, QK, f"wqk{i}"))
        wout = []
        for i in range(5):
            cw = 121 if i == 4 else 120
            wout.append(load_fr(wout_aug[120 * i:120 * i + cw, :], cw, DIM,
                                f"wout{i}"))

        mask = []
        maskf = []
        for i in range(4):
            tf = P1.tile([125, S], f32, tag=f"maskf{i}")
            nc.gpsimd.dma_start(tf[:], mask2T[125 * i:125 * (i + 1), :])
            maskf.append(tf)
            # fold the (1/S)^2 of relu(sim/S)^2 into the mask; bf16 for 2x TT
            t = P1.tile([125, S], bf16, tag=f"mask{i}")
            nc.vector.tensor_scalar_mul(t[:], tf[:], 1.0 / (S * S))
            mask.append(t)

        cos2 = P1.tile([64, S], bf16)
        nc.scalar.dma_start(cos2[:], cos2_d[:, :])
        sin2 = P1.tile([64, S], bf16)
        nc.scalar.dma_start(sin2[:], sin2_d[:, :])
        p64 = P1.tile([64, 64], bf16)
        nc.scalar.dma_start(p64[:], p64_d[:, :])
        eccnt = P1.tile([120, 24], bf16)
        nc.scalar.dma_start(eccnt[:], eccnt_d[:, :])
        eczaug = P1.tile([25, 120], bf16)
        nc.scalar.dma_start(eczaug[:], eczaug_d[:, :])
        iota16 = P1.tile([128, 16], f32)
        nc.scalar.dma_start(iota16[:], bcast_ap(iota16_d, 128))
        eps_t = P1.tile([125, 1], f32)
        nc.vector.memset(eps_t[:], EPS)

        def fill_fr(out_ap, value, src_ap):
            # constant-fill a float32r region via DVE (memset lacks an fr
            # value type; fp32r matmul operands must be engine-written anyway)
            nc.vector.tensor_scalar(out_ap, src_ap, 0.0, float(value),
                                    ALU.mult, ALU.add)

        gch, bch = [], []
        for i, (c0, cw, _) in enumerate(CHUNKS):
            g = P1.tile([cw, 1], f32, tag=f"g{i}")
            nc.gpsimd.dma_start(g[:], ln_g[c0:c0 + cw, None])
            gch.append(g)
            b = P1.tile([cw, 1], f32, tag=f"b{i}")
            nc.gpsimd.dma_start(b[:], ln_b[c0:c0 + cw, None])
            bch.append(b)
        gam = P1.tile([QK, 2], f32)
        nc.gpsimd.dma_start(gam[:, 0:1], gamma[0, :, None])
        nc.gpsimd.dma_start(gam[:, 1:2], gamma[1, :, None])
        bet = P1.tile([QK, 2], f32)
        nc.gpsimd.dma_start(bet[:, 0:1], beta[0, :, None])
        nc.gpsimd.dma_start(bet[:, 1:2], beta[1, :, None])

        # t5 bias diag: [1,999] = rel_emb.T @ onehotT, then Toeplitz reads
        emb_sb = P1.tile([NB, 1], f32)
        nc.scalar.dma_start(emb_sb[:], rel_emb[:, :])
        oh_sb = P1.tile([NB, 999], f32)
        nc.scalar.dma_start(oh_sb[:], onehotT[:, :])
        dg_ps0 = psum([1, 512])
        dg_ps1 = psum([1, 487])
        nc.tensor.matmul(dg_ps0[:], emb_sb[:], oh_sb[:, 0:512],
                         start=True, stop=True)
        nc.tensor.matmul(dg_ps1[:], emb_sb[:], oh_sb[:, 512:999],
                         start=True, stop=True)
        diag_sb = P1.tile([1, 999], f32)
        nc.vector.memset(diag_sb[:], 0.0)
        nc.scalar.activation(diag_sb[:, 0:512], dg_ps0[:], ACT.Identity,
                             bias=0.0)
        nc.scalar.activation(diag_sb[:, 512:999], dg_ps1[:], ACT.Identity,
                             bias=0.0)
        biasT = []
        for jt in range(4):
            t = P1.tile([125, S], f32, tag=f"biasT{jt}")
            # T[jj, m] = diag_rev[125*jt + jj + m] via SBUF-source expansion
            # DMA (partition-0 vector fanned out); consumers read m reversed
            src = bass.AP(tensor=diag_sb.tensor,
                          offset=diag_sb.offset + 125 * jt,
                          ap=[list(diag_sb.ap[0]), [1, 125], [1, S]])
            nc.sync.dma_start(t[:], src)
            nc.vector.tensor_scalar_mul(t[:], t[:], SQRT_QK)
            biasT.append(t)

        # bv75 tiles: row 24 is the constant "+0.25 pattern" rhs row
        bv75 = []
        for ctn in range(5):
            t = P1.tile([25, 100], bf16, tag=f"bv{ctn}")
            nc.vector.memset(t[:], 1.0)
            bv75.append(t)

        # ---------------- per-batch
        for b in range(NB_PER_CORE):
            r0 = b * S

            xb, nxp = [], []
            for st in range(4):
                t = PB2.tile([125, DIM], f32, tag=f"xb{st}")
                nc.sync.dma_start(t[:], x3[r0 + 125 * st:r0 + 125 * (st + 1), :])
                xb.append(t)

            # LayerNorm (s-major)
            for st in range(4):
                stt_ = PB1.tile([125, 6], f32, tag="bnst")
                nc.vector.bn_stats(stt_[:], xb[st][:])
                mv = PB1.tile([125, 2], f32, tag="bnmv")
                nc.vector.bn_aggr(mv[:], stt_[:])
                sq = PB1.tile([125, 1], f32, tag="bnsq")
                nc.scalar.activation(sq[:], mv[:, 1:2], ACT.Sqrt, bias=eps_t[:])
                nc.vector.reciprocal(sq[:], sq[:])
                t = PBY.tile([125, DIM], f32, tag=f"nxp{st}")
                nc.vector.tensor_scalar(t[:], xb[st][:], mv[:, 0:1], sq[:, 0:1],
                                        ALU.subtract, ALU.mult)
                nxp.append(t)

            # transpose to channel-major; token shift via column-offset views
            nx2Tv = []
            for i, (c0, cw, shifted) in enumerate(CHUNKS):
                last = i == len(CHUNKS) - 1
                tp = psum([cw, S], tag="ps")
                for st in range(4):
                    nc.tensor.transpose(tp[:, 125 * st:125 * (st + 1)],
                                        nxp[st][:, c0:c0 + cw],
                                        ident[:125, :125])
                tw = cw + 1 if last else cw
                t = PB2.tile([tw, S + 1], fr32, tag=f"nx2T{i}")
                if last:
                    fill_fr(t[:, 0:S], 1.0, maskf[0][0:tw, 0:S])
                    fill_fr(t[:, S:S + 1], 1.0, eps_t[0:tw, 0:1])
                nc.scalar.activation(t[0:cw, 1:S + 1], tp[:], ACT.Identity,
                                     bias=bch[i][:], scale=gch[i][:])
                if shifted:
                    fill_fr(t[0:cw, 0:1], 0.0, iota16[0:cw, 0:1])
                    nx2Tv.append(t[0:cw, 0:S])
                elif last:
                    nx2Tv.append(t[0:cw + 1, 1:S + 1])
                else:
                    nx2Tv.append(t[0:cw, 1:S + 1])

            def silu(out, ps_in, tagbase):
                if _SILU_NATIVE:
                    nc.scalar.activation(out, ps_in, ACT.Silu)
                else:
                    sg = PB1.tile([128, S], f32, tag="sigtmp")
                    p = ps_in.shape[0]
                    fn = ps_in.free_size()
                    nc.scalar.activation(sg[0:p, 0:fn], ps_in, ACT.Sigmoid)
                    nc.vector.tensor_mul(out, sg[0:p, 0:fn], ps_in)

            # h GEMM: v (s-major) and gateT (c-major), silu fused in evac
            v_sb = []
            for st in range(4):
                t = PB2.tile([125, HID], bf16, tag=f"v{st}")
                for half in range(2):
                    vp = psum([125, 300], tag="ps")
                    for i in range(4):
                        nc.tensor.matmul(
                            vp[:],
                            nx2Tv[i][:, 125 * st:125 * (st + 1)],
                            wh[i][:, 300 * half:300 * (half + 1)],
                            start=(i == 0), stop=(i == 3))
                    silu(t[:, 300 * half:300 * (half + 1)], vp[:], "v")
                v_sb.append(t)

            gateT = []
            for ctn in range(5):
                gp = psum([120, S], tag="ps")
                for i in range(4):
                    nc.tensor.matmul(
                        gp[:],
                        wh[i][:, HID + 120 * ctn:HID + 120 * (ctn + 1)],
                        nx2Tv[i],
                        start=(i == 0), stop=(i == 3))
                t = PB2.tile([120, S], f32, tag=f"gateT{ctn}")
                silu(t[:], gp[:], "g")
                gateT.append(t)

            # qk path
            qk_ps = psum([QK, S], tag="ps")
            for i in range(4):
                nc.tensor.matmul(qk_ps[:],
                                 wqk[i][:], nx2Tv[i],
                                 start=(i == 0), stop=(i == 3))
            qkT = PB1.tile([QK, S], f32, tag="qkT")
            silu(qkT[:], qk_ps[:], "qk")

            QK2 = PB1.tile([64, S], bf16, tag="QK2")
            nc.scalar.activation(QK2[0:32, :], qkT[0:32, :], ACT.Identity,
                                 bias=bet[0:32, 0:1], scale=gam[0:32, 0:1])
            nc.scalar.activation(QK2[32:64, :], qkT[0:32, :], ACT.Identity,
                                 bias=bet[0:32, 1:2], scale=gam[0:32, 1:2])
            q_sb = PB1.tile([QK, S], bf16, tag="q_sb")
            k_sb = PB1.tile([QK, S], bf16, tag="k_sb")
            for (lo, hi) in ((32, 64), (64, 128)):
                nc.scalar.activation(q_sb[lo:hi, :], qkT[lo:hi, :], ACT.Identity,
                                     bias=bet[lo:hi, 0:1], scale=gam[lo:hi, 0:1])
                nc.scalar.activation(k_sb[lo:hi, :], qkT[lo:hi, :], ACT.Identity,
                                     bias=bet[lo:hi, 1:2], scale=gam[lo:hi, 1:2])
            perm_ps = psum([64, S], tag="ps")
            nc.tensor.matmul(perm_ps[:], p64[:], QK2[:],
                             start=True, stop=True)
            rot1 = PB1.tile([64, S], bf16, tag="rot1")
            nc.vector.tensor_mul(rot1[:], QK2[:], cos2[:])
            rot2 = PB1.tile([64, S], bf16, tag="rot2")
            nc.vector.tensor_mul(rot2[:], perm_ps[:], sin2[:])
            nc.vector.tensor_add(q_sb[0:32, :], rot1[0:32, :], rot2[0:32, :])
            nc.vector.tensor_add(k_sb[0:32, :], rot1[32:64, :], rot2[32:64, :])

            # attention scores -> attnT
            attnT = []
            for jt in range(4):
                sim_ps = psum([125, S], tag="ps")
                nc.tensor.matmul(sim_ps[:],
                                 k_sb[:, 125 * jt:125 * (jt + 1)],
                                 q_sb[:], start=True, stop=True)
                tsb = PB1.tile([125, S], f32, tag="tsb")
                bT = biasT[jt]
                brev = bass.AP(tensor=bT.tensor, offset=bT.offset + (S - 1),
                               ap=[bT.ap[0], [-1, S]])
                nc.vector.tensor_tensor(tsb[:], brev, sim_ps[:], ALU.add)
                u = PB1.tile([125, S], bf16, tag="u")
                nc.vector.scalar_tensor_tensor(u[:], tsb[:], 0.0, tsb[:],
                                               ALU.max, ALU.mult)
                t = PB2.tile([125, S], bf16, tag=f"attnT{jt}")
                nc.vector.tensor_mul(t[:], u[:], mask[jt][:])
                attnT.append(t)

            # gate mask statistics (trim = 1)
            counts = PB1.tile([128, 200], f32, tag="counts")
            nc.vector.memset(counts[:], 0.0)
            for ctn in range(5):
                g = gateT[ctn]
                ind4 = PB1.tile([120, 400], f32, tag="ind4")
                gv = bass.AP(tensor=g.tensor, offset=g.offset,
                             ap=[g.ap[0], [5, 100], [1, 4]])
                nc.vector.tensor_scalar(ind4[:], gv, 1.0, 0.0,
                                        ALU.is_ge, ALU.add)
                colred = PB1.tile([120, 100], bf16, tag="colred")
                iv = bass.AP(tensor=ind4.tensor, offset=ind4.offset,
                             ap=[ind4.ap[0], [4, 100], [1, 4]])
                with nc.allow_low_precision("counts are small ints, exact in bf16"):
                    nc.vector.tensor_reduce(colred[:], iv,
                                            mybir.AxisListType.X, ALU.add)
                cnt_ps = psum([24, 100], tag="ps")
                nc.tensor.matmul(cnt_ps[:], eccnt[:], colred[:],
                                 start=True, stop=True)
                if ctn < 4:
                    nc.scalar.activation(counts[32 * ctn:32 * ctn + 24, 0:100],
                                         cnt_ps[:], ACT.Identity, bias=0.0)
                else:
                    nc.scalar.activation(counts[0:24, 100:200], cnt_ps[:],
                                         ACT.Identity, bias=0.0)

            cm1 = PB1.tile([128, 1], f32, tag="cm1")
            nc.vector.tensor_reduce(cm1[:], counts[:],
                                    mybir.AxisListType.X, ALU.max)
            cmaxP = PB1.tile([128, 1], f32, tag="cmaxP")
            nc.gpsimd.partition_all_reduce(cmaxP[:], cm1[:], channels=128,
                                           reduce_op=bass_isa.ReduceOp.max)
            c2p = PB1.tile([128, 16], f32, tag="c2p")
            scr = PB1.tile([128, 200], f32, tag="scr")
            for t_ in range(1, 17):
                nc.vector.tensor_scalar(scr[:], counts[:], float(t_), 0.0,
                                        ALU.is_ge, ALU.add,
                                        accum_out=c2p[:, t_ - 1:t_])
            c2P = PB1.tile([128, 16], f32, tag="c2P")
            nc.gpsimd.partition_all_reduce(c2P[:], c2p[:], channels=128,
                                           reduce_op=bass_isa.ReduceOp.add)
            v1t = PB1.tile([128, 16], f32, tag="v1t")
            nc.vector.tensor_scalar(v1t[:], c2P[:], 3600.0, 0.0,
                                    ALU.is_gt, ALU.add)
            v2t = PB1.tile([128, 16], f32, tag="v2t")
            nc.vector.scalar_tensor_tensor(v2t[:], iota16[:], cmaxP[:, 0:1],
                                           v1t[:], ALU.is_le, ALU.mult)
            nc.vector.tensor_mul(v2t[:], v2t[:], iota16[:])
            tmax = PB1.tile([128, 1], f32, tag="tmax")
            nc.vector.tensor_reduce(tmax[:], v2t[:],
                                    mybir.AxisListType.X, ALU.max)
            e0 = PB1.tile([128, 1], f32, tag="e0")
            nc.vector.tensor_scalar(e0[:], tmax[:], 0.0, 0.0,
                                    ALU.is_equal, ALU.add)
            t2P = PB1.tile([128, 1], f32, tag="t2P")
            nc.vector.scalar_tensor_tensor(t2P[:], e0[:], cmaxP[:, 0:1],
                                           tmax[:], ALU.mult, ALU.add)
            for ctn in range(5):
                if ctn < 4:
                    cw_ = counts[32 * ctn:32 * ctn + 24, 0:100]
                    tw_ = t2P[32 * ctn:32 * ctn + 24, 0:1]
                else:
                    cw_ = counts[0:24, 100:200]
                    tw_ = t2P[0:24, 0:1]
                nc.vector.tensor_scalar(bv75[ctn][0:24, :], cw_, tw_, 0.75,
                                        ALU.is_ge, ALU.mult)

            # attnout (c-major) and z = attnoutT * gateT * gm
            z = []
            for ctn in range(5):
                ao_ps = psum([120, S], tag="ps")
                for jt in range(4):
                    nc.tensor.matmul(
                        ao_ps[:],
                        v_sb[jt][:, 120 * ctn:120 * (ctn + 1)],
                        attnT[jt][:],
                        start=(jt == 0), stop=(jt == 3))
                gm_ps = psum([120, 100], tag="ps")
                nc.tensor.matmul(gm_ps[:], eczaug[:], bv75[ctn][:],
                                 start=True, stop=True)
                agt = PB1.tile([120, S], f32, tag="agt")
                nc.vector.tensor_mul(agt[:], ao_ps[:], gateT[ctn][:])
                tw = 121 if ctn == 4 else 120
                t = PB2.tile([tw, S], fr32, tag=f"z{ctn}")
                if ctn == 4:
                    fill_fr(t[:], 1.0, maskf[0][0:tw, 0:S])
                gmb = bass.AP(tensor=gm_ps.tensor, offset=gm_ps.offset,
                              ap=[gm_ps.ap[0], [1, 100], [0, 5]])
                nc.vector.tensor_mul(t[0:120, :], agt[:], gmb)
                zv = bass.AP(tensor=t.tensor, offset=t.offset + 4,
                             ap=[[t.ap[0][0], 120], [5, 100]])
                fill_fr(zv, 0.0, maskf[0][0:120, 0:100])
                z.append(t)

            # y = z @ Wout (+bout row) + x
            for st in range(4):
                y_ps = psum([125, DIM], tag="ps")
                for ctn in range(5):
                    nc.tensor.matmul(
                        y_ps[:],
                        z[ctn][:, 125 * st:125 * (st + 1)],
                        wout[ctn][:],
                        start=(ctn == 0), stop=(ctn == 4))
                ty = PB2.tile([125, DIM], f32, tag=f"y{st}")
                nc.vector.tensor_add(ty[:], y_ps[:], xb[st][:])
                nc.sync.dma_start(
                    y3[r0 + 125 * st:r0 + 125 * (st + 1), :], ty[:])

    nc.compile()
    return nc


# ------------------------------------------------------------------- kernel()
def _stage_core_inputs(inputs, batches, consts):
    x = np.asarray(inputs["x"], np.float32)
    xs = x[batches]                                   # [3, 500, 300]
    m = {
        "x3": np.ascontiguousarray(xs.reshape(NB_PER_CORE * S, DIM)),
        "wh_aug": np.ascontiguousarray(np.vstack(
            [np.asarray(inputs["Wh"], np.float32),
             np.asarray(inputs["bh"], np.float32)[None, :]])),
        "wqk_aug": np.ascontiguousarray(np.vstack(
            [np.asarray(inputs["Wqk"], np.float32),
             np.asarray(inputs["bqk"], np.float32)[None, :]])),
        "wout_aug": np.ascontiguousarray(np.vstack(
            [np.asarray(inputs["Wout"], np.float32),
             np.asarray(inputs["bout"], np.float32)[None, :]])),
        "gamma": np.asarray(inputs["gamma"], np.float32),
        "beta": np.asarray(inputs["beta"], np.float32),
        "rel_emb": np.asarray(inputs["rel_emb"], np.float32),
        "ln_g": np.asarray(inputs["ln_g"], np.float32),
        "ln_b": np.asarray(inputs["ln_b"], np.float32),
        "mask2T": np.ascontiguousarray(
            np.asarray(inputs["my_mask2"], np.float32).T),
    }
    m.update(consts)
    return m


def kernel(**inputs):
    from concourse.bass_utils import run_bass_kernel_spmd

    if "nc" not in _CACHE:
        _CACHE["nc"] = _build_program()
    nc = _CACHE["nc"]

    consts = _host_consts()
    core_batches = []
    for c in range(4):
        core_batches.append([3 * c, 3 * c + 1, 3 * c + 2])
    for c in range(4):
        b0 = 12 + 2 * c
        core_batches.append([b0, b0 + 1, b0])      # 3rd is a dropped duplicate

    in_maps = [_stage_core_inputs(inputs, cb, consts) for cb in core_batches]
    res = run_bass_kernel_spmd(nc, in_maps, list(range(NCORES)))

    out = np.empty((B, S, DIM), np.float32)
    for c in range(NCORES):
        y = res.results[c]["y3"].reshape(NB_PER_CORE, S, DIM)
        take = 3 if c < 4 else 2
        for k in range(take):
            out[core_batches[c][k]] = y[k]
    return out
